# revision 14
# baseline (speedup 1.0000x reference)
"""Trainium2 Bass kernel for nn_CCGGenerator (LSTM encoder + attention decoder).

Sharding: data-parallel, batch 128 -> 16 per core across 8 cores.
All weights replicated. Self-contained; everything hardcoded.

v2 design (per core, B=16):
- Encoder gates computed transposed: gates.T [2048, 16] as 16 PSUM cols-of-16
  split into TWO per-group PSUM tiles [128, 128] (group g = hidden half).
  Gate m-chunk order is re-packed in prep to
    [G0: i0 i1 f0 f1 o0 o1 g0 g1 | G1: i2 i3 f2 f3 o2 o3 g2 g3]
  so each group's gates are contiguous 128 PSUM cols: [i(32) f(32) o(32) g(32)].
- g-gate rows are pre-scaled x2 so tanh(x) = 2*sigmoid(2x)-1 turns the whole
  gate nonlinearity into ONE sigmoid over 128 cols + a tiny affine on 32 cols.
- Tail per group (8 wide ops instead of ~23 narrow ones):
    DVE  add   ga = gps_g + xg_g            [128,128]
    ACT  sigm  ga = sigmoid(ga)             [128,128] (g cols pre-scaled)
    DVE  affn  gt = 2*sigm-1                [128,32]
    DVE  ig    = i * gt                     [128,32]
    GPS  fc    = f * c_prev                 [128,32]   (parallel w/ DVE)
    DVE  c     = ig + fc                    [128,32]
    ACT  th    = tanh(c)                    [128,32]
    DVE  h     = o * th -> cl_sb (bf16)     [128,32]
- Software pipeline: PE emits S1=G0 MMs, S2=G1 MMs; tail(G0) overlaps S2 and
  next step's S1 (which only needs h chunks k=0,1 produced by tail(G0)).
- xg = x @ Wih.T + bias precomputed per 8-step chunk, stored s-major
  [128, s*256 + m*16 + b] so the tail add is a flat 2-level AP.
- Decoder: same structure, c = i*g (h0=c0=0 per reference), bias instead of xg.
- Attention per b unchanged from v1 (scores via strided CL reads, PE
  transposes for CL_b, ctx matmul, fused leaky-relu output GEMM).
"""
import sys
sys.path.insert(0, "/opt/trn_rl_repo")

import numpy as np
import ml_dtypes
from contextlib import ExitStack

import concourse.bass as bass
import concourse.tile as tile
from concourse import bacc, mybir
from concourse.bass_utils import run_bass_kernel_spmd

F32 = mybir.dt.float32
BF16 = mybir.dt.bfloat16
AF = mybir.ActivationFunctionType
OP = mybir.AluOpType
BF = ml_dtypes.bfloat16

NCORES = 8
B = 16          # batch per core
N_STEPS = 1024  # encoder sequence length
SCH = 8         # steps per xg chunk
C = 32
H = 512
G = 2048        # 4H
T = 24
KH = 4          # hidden 128-chunks
M16 = 16        # gate-dim 128-chunks


def build_program(n_steps=N_STEPS):
    nch = n_steps // SCH
    nac = n_steps // 128  # attention n-chunks
    nc = bacc.Bacc("TRN2", target_bir_lowering=False, debug=False,
                   num_devices=NCORES)

    p_cond = nc.declare_dram_parameter("cond_aT", [C + 1, n_steps * B], BF16, isOutput=False)
    p_wih = nc.declare_dram_parameter("wihT_a", [C + 1, G], BF16, isOutput=False)
    p_whh = nc.declare_dram_parameter("whhT", [128, 64 * 128], BF16, isOutput=False)
    p_wcell = nc.declare_dram_parameter("wcellT", [128, 64 * 128], BF16, isOutput=False)
    p_bcell = nc.declare_dram_parameter("bcell_bc", [128, 256], F32, isOutput=False)
    p_wout = nc.declare_dram_parameter("woutT", [128, 8 * C], BF16, isOutput=False)
    p_bout = nc.declare_dram_parameter("bout_bc", [T, C], F32, isOutput=False)
    p_id = nc.declare_dram_parameter("ident", [128, 128], BF16, isOutput=False)
    p_idf = nc.declare_dram_parameter("identf", [128, 128], F32, isOutput=False)
    p_out = nc.declare_dram_parameter("out", [B, T, C], F32, isOutput=True)

    with tile.TileContext(nc) as tc, ExitStack() as ctx:
        const = ctx.enter_context(tc.tile_pool(name="const", bufs=1))

        cl_sb = const.tile([128, n_steps * KH * B], BF16, tag="cl_sb")  # 16 MB
        hd_sb = const.tile([128, T * KH * B], BF16, tag="hd_sb")       # t-major
        wout_sb = const.tile([128, 8 * C], BF16, tag="wout_sb")
        nc.sync.dma_start(wout_sb[:], p_wout[:])
        bout_sb = const.tile([T, C], F32, tag="bout_sb")
        nc.sync.dma_start(bout_sb[:], p_bout[:])
        id_sb = const.tile([128, 128], BF16, tag="id_sb")
        nc.sync.dma_start(id_sb[:], p_id[:])
        idf_sb = const.tile([128, 128], F32, tag="idf_sb")
        nc.sync.dma_start(idf_sb[:], p_idf[:])
        bcell_sb = const.tile([128, 256], F32, tag="bcell_sb")
        nc.sync.dma_start(bcell_sb[:], p_bcell[:])

        cl3 = cl_sb[:].rearrange("p (n k b) -> p n k b", k=KH, b=B)
        hd_v = hd_sb[:].rearrange("p (t k b) -> p t k b", k=KH, b=B)

        def lstm_tail(g, gps_g, xg_g, c_f, h_out, st_pool, ga_pool, fc_pool):
            """Per-group tail. gps_g [128,128] PSUM (cols i f o gt, 32 each).
            xg_g: [128,128] AP to add (xg slice or bias slice).
            c_f: persistent cell state [128, 64] or None (decoder: c = i*g).
            h_out: [128,32] AP (bf16 dest)."""
            ga = ga_pool.tile([128, 128], F32, tag=f"ga{g}")
            nc.scalar.activation(ga[:, 0:96], gps_g[:, 0:96], AF.Sigmoid)
            nc.scalar.activation(ga[:, 96:128], gps_g[:, 96:128], AF.Tanh)
            ig = st_pool.tile([128, 32], F32, tag=f"ig{g}")
            if c_f is not None:
                fc = fc_pool.tile([128, 32], F32, tag=f"fc{g}")
                nc.gpsimd.tensor_tensor(fc[:], ga[:, 32:64],
                                        c_f[:, g * 32:(g + 1) * 32], op=OP.mult)
                nc.vector.tensor_tensor(ig[:], ga[:, 0:32], ga[:, 96:128], op=OP.mult)
                nc.vector.tensor_tensor(c_f[:, g * 32:(g + 1) * 32], ig[:], fc[:],
                                        op=OP.add)
                csrc = c_f[:, g * 32:(g + 1) * 32]
            else:
                nc.vector.tensor_tensor(ig[:], ga[:, 0:32], ga[:, 96:128], op=OP.mult)
                csrc = ig[:]
            th = st_pool.tile([128, 32], F32, tag=f"th{g}")
            nc.scalar.activation(th[:], csrc, AF.Tanh)
            nc.vector.tensor_tensor(h_out, ga[:, 64:96], th[:], op=OP.mult)

        # ---------------- encoder + decoder (shared gate/state pools) ----------------
        rec_pools = ExitStack()
        gps_pool = rec_pools.enter_context(tc.tile_pool(name="gps", bufs=2, space="PSUM"))
        ga_pool = rec_pools.enter_context(tc.tile_pool(name="gtmp", bufs=2))
        st_pool = rec_pools.enter_context(tc.tile_pool(name="st", bufs=2))
        fc_pool = rec_pools.enter_context(tc.tile_pool(name="fcp", bufs=2))

        def step_mms(gps2, whh_like, hin, addv):
            """Emit the gate MMs for one step, grouped G0 then G1. Each
            group's psum bank holds ONE accumulation group: an identity
            matmul first injects xg (or the bias) over all 128 cols and opens
            the group, then the 32 Whh MMs accumulate in k-major order (so
            the k{0,1} half of step t+1 only depends on h chunks 0,1, i.e.
            tail g0 of step t)."""
            for g in (0, 1):
                nc.tensor.matmul(gps2[g][:], idf_sb[:],
                                 addv(g), start=True, stop=False)
                for k in range(KH):
                    for mj in range(8):
                        m = g * 8 + mj
                        nc.tensor.matmul(
                            gps2[g][:, bass.ts(mj, B)],
                            whh_like[:, bass.ts(m * KH + k, 128)],
                            hin(k), start=False,
                            stop=(k == KH - 1 and mj == 7))

        with tc.tile_pool(name="enc", bufs=1) as enc_pool, \
             tc.tile_pool(name="xg", bufs=2) as xg_pool, \
             tc.tile_pool(name="xg_ps", bufs=2, space="PSUM") as xg_ps_pool:
            whh_sb = enc_pool.tile([128, 64 * 128], BF16, tag="whh_sb")
            nc.sync.dma_start(whh_sb[:], p_whh[:])
            wih_sb = enc_pool.tile([C + 1, G], BF16, tag="wih_sb")
            nc.sync.dma_start(wih_sb[:], p_wih[:])
            c_f = enc_pool.tile([128, KH * B], F32, tag="c_f")
            h0 = enc_pool.tile([128, KH * B], BF16, tag="h0")
            nc.any.memset(c_f[:], 0.0)
            nc.any.memset(h0[:], 0.0)

            def fetch_cond(j):
                t = xg_pool.tile([C + 1, SCH * B], BF16, tag="cond_ch")
                nc.sync.dma_start(t[:], p_cond[:, bass.ts(j, SCH * B)])
                return t

            def xg_alloc():
                t = xg_pool.tile([128, SCH * M16 * B], F32, tag="xg_sb")
                return t, t[:].rearrange("p (s m b) -> p s m b", s=SCH, m=M16)

            def xg_compute(xg_v, cond_t, mpair):
                """Two gate m-chunks (2*mpair, 2*mpair+1) of xg for a whole
                8-step chunk: two MMs into one PSUM tile, one DVE copy out
                (GPSIMD cannot read PSUM)."""
                xps = xg_ps_pool.tile([128, 2 * SCH * B], F32, tag="xps")
                for i in (0, 1):
                    nc.tensor.matmul(xps[:, bass.ts(i, SCH * B)],
                                     wih_sb[:, bass.ts(2 * mpair + i, 128)],
                                     cond_t[:], start=True, stop=True)
                nc.vector.tensor_copy(
                    xg_v[:, :, 2 * mpair:2 * mpair + 2, :],
                    xps[:].rearrange("p (m s b) -> p s m b", m=2, s=SCH))

            # prologue: cond 0,1 in flight; xg[0] computed in a burst
            cond_cur = fetch_cond(0)
            cond_nxt = fetch_cond(1) if nch > 1 else None
            xg_cur, xgv_cur = xg_alloc()
            for mp in range(M16 // 2):
                xg_compute(xgv_cur, cond_cur, mp)

            for chv in range(nch):
                if chv + 2 < nch:
                    cond_fut = fetch_cond(chv + 2)
                else:
                    cond_fut = None
                if chv + 1 < nch:
                    xg_nxt, xgv_nxt = xg_alloc()
                for s in range(SCH):
                    n = chv * SCH + s
                    hin = (lambda k: h0[:, bass.ts(k, B)]) if n == 0 else \
                        (lambda k, _n=n: cl_sb[:, (_n - 1) * 64 + k * B:(_n - 1) * 64 + (k + 1) * B])
                    gps2 = [gps_pool.tile([128, 128], F32, tag=f"gps{g}",
                                           name=f"gps{g}") for g in (0, 1)]
                    step_mms(gps2, whh_sb, hin,
                             lambda g, _s=s: xg_cur[:, _s * 256 + g * 128:_s * 256 + (g + 1) * 128])
                    # next chunk's xg precompute: fills the PE's wait-for-h gap
                    if chv + 1 < nch:
                        xg_compute(xgv_nxt, cond_nxt, s)
                    for g in (0, 1):
                        lstm_tail(g, gps2[g][:], None,
                                  c_f,
                                  cl_sb[:, n * 64 + g * 32:n * 64 + (g + 1) * 32],
                                  st_pool, ga_pool, fc_pool)
                if chv + 1 < nch:
                    xg_cur, xgv_cur = xg_nxt, xgv_nxt
                    cond_cur, cond_nxt = cond_nxt, cond_fut

        # ---------------- decoder ----------------
        with tc.tile_pool(name="dec", bufs=1) as dec_pool:
            wcell_sb = dec_pool.tile([128, 64 * 128], BF16, tag="wcell_sb")
            nc.sync.dma_start(wcell_sb[:], p_wcell[:])
            for t in range(T):
                if t == 0:
                    hin = lambda k: cl_sb[:, (n_steps - 1) * 64 + k * B:(n_steps - 1) * 64 + (k + 1) * B]
                else:
                    hin = lambda k, _t=t: hd_sb[:, (_t - 1) * 64 + k * B:(_t - 1) * 64 + (k + 1) * B]
                gps2 = [gps_pool.tile([128, 128], F32, tag=f"gps{g}",
                                       name=f"gps{g}") for g in (0, 1)]
                step_mms(gps2, wcell_sb, hin,
                         lambda g: bcell_sb[:, g * 128:(g + 1) * 128])
                for g in (0, 1):
                    lstm_tail(g, gps2[g][:], None,
                              None,
                              hd_sb[:, t * 64 + g * 32:t * 64 + (g + 1) * 32],
                              st_pool, ga_pool, fc_pool)
        rec_pools.close()

        # ---------------- attention + output, per batch ----------------
        with tc.tile_pool(name="att_fix", bufs=2) as att_fix, \
             tc.tile_pool(name="scr_ps", bufs=1, space="PSUM") as scr_ps_pool, \
             tc.tile_pool(name="tp_ps", bufs=2, space="PSUM") as tp_ps_pool, \
             tc.tile_pool(name="ctx_ps", bufs=2, space="PSUM") as ctx_ps_pool:
            for b in range(B):
                # scores [24, n]: lhsT = hd strided, rhs = cl_sb strided (CL.T native)
                scr = scr_ps_pool.tile([T, n_steps], F32, tag="scr")
                scn = min(512, n_steps)
                for k in range(KH):
                    for j in range(n_steps // scn):
                        rhs = cl3[:, j * scn:(j + 1) * scn, k, b]
                        nc.tensor.matmul(scr[:, bass.ts(j, scn)], hd_v[:, :, k, b],
                                         rhs, start=(k == 0), stop=(k == KH - 1))
                nmx = att_fix.tile([T, 1], F32, tag="nmx")
                nc.vector.reduce_max(nmx[:], scr[:], axis=mybir.AxisListType.X, negate=True)
                ex = att_fix.tile([T, n_steps], F32, tag="ex")
                sm = att_fix.tile([T, 1], F32, tag="sm")
                nc.scalar.activation(ex[:], scr[:], AF.Exp, bias=nmx[:], accum_out=sm[:])
                rc = att_fix.tile([T, 1], F32, tag="rc")
                nc.vector.reciprocal(rc[:], sm[:])
                cof = att_fix.tile([T, n_steps], BF16, tag="cof")
                nc.vector.tensor_scalar(cof[:], ex[:], rc[:], None, op0=OP.mult)
                # coeff.T [n, 24] via PE transposes
                cT = att_fix.tile([128, nac * T], BF16, tag="cT")
                for j in range(nac):
                    tp = tp_ps_pool.tile([128, 128], BF16, tag="tp")
                    nc.tensor.transpose(tp[:, 0:T], cof[:, bass.ts(j, 128)], id_sb[0:T, 0:T])
                    nc.vector.tensor_copy(cT[:, bass.ts(j, T)], tp[:, 0:T])
                # CL_b n-partitioned tiles via PE transposes
                clb = att_fix.tile([128, nac * KH * 128], BF16, tag="clb")
                for j in range(nac):
                    for k in range(KH):
                        tpc = tp_ps_pool.tile([128, 128], BF16, tag="tp")
                        nc.tensor.transpose(tpc[:], cl3[:, j * 128:(j + 1) * 128, k, b],
                                            id_sb[:, :])
                        nc.vector.tensor_copy(clb[:, bass.ts(j * KH + k, 128)], tpc[:])
                # ctx.T [512, 24]
                ctxp = ctx_ps_pool.tile([128, KH * T], F32, tag="ctxp")
                for k in range(KH):
                    for j in range(nac):
                        nc.tensor.matmul(ctxp[:, bass.ts(k, T)],
                                         clb[:, bass.ts(j * KH + k, 128)],
                                         cT[:, bass.ts(j, T)],
                                         start=(j == 0), stop=(j == nac - 1))
                # out [24, 32]
                ob_ps = scr_ps_pool.tile([T, C], F32, tag="ob_ps")
                for jj in range(8):
                    lr = att_fix.tile([128, T], BF16, tag="lr")
                    src = hd_v[:, :, jj, b] if jj < KH else ctxp[:, bass.ts(jj - KH, T)]
                    nc.scalar.activation(lr[:], src, AF.Lrelu, alpha=0.01)
                    nc.tensor.matmul(ob_ps[:], lr[:], wout_sb[:, bass.ts(jj, C)],
                                     start=(jj == 0), stop=(jj == 7))
                ob = att_fix.tile([T, C], F32, tag="ob")
                nc.vector.tensor_tensor(ob[:], ob_ps[:], bout_sb[:], op=OP.add)
                nc.sync.dma_start(p_out[b], ob[:])
    nc.compile()
    return nc


# m-chunk permutation: new m-position -> original gate chunk index (of 16).
# Original chunks: i=0..3, f=4..7, g=8..11, o=12..15 (PyTorch i,f,g,o order).
# New order: [G0: i0 i1 f0 f1 o0 o1 g0 g1 | G1: i2 i3 f2 f3 o2 o3 g2 g3]
M_PERM = [0, 1, 4, 5, 12, 13, 8, 9,
          2, 3, 6, 7, 14, 15, 10, 11]


def _reorder_cols(Wt):
    """Wt [*, 2048]: permute gate columns into the new m-chunk order."""
    Wn = Wt.reshape(Wt.shape[0], 16, 128)[:, M_PERM, :]
    return np.ascontiguousarray(Wn).reshape(Wt.shape[0], 2048)


def prep_inputs(condition, Wih_enc, Whh_enc, bih_enc, bhh_enc,
                Wih_cell, Whh_cell, bih_cell, bhh_cell, W_out, b_out,
                n_steps=N_STEPS):
    def tile64(Wt):  # [512, 2048] -> [128, 64*128], col block m*4+k
        return np.ascontiguousarray(
            Wt.reshape(KH, 128, M16, 128).transpose(1, 2, 0, 3).reshape(128, 64 * 128)).astype(BF)

    whhT = tile64(_reorder_cols(Whh_enc.T.astype(np.float32)))
    wcellT = tile64(_reorder_cols(Wih_cell.T.astype(np.float32)))
    wih_a = np.zeros((C + 1, G), np.float32)
    wih_a[0:C] = Wih_enc.T
    wih_a[C] = bih_enc + bhh_enc
    wih_a = _reorder_cols(wih_a)
    bias_c = _reorder_cols((bih_cell + bhh_cell).astype(np.float32)[None, :])[0]
    bcell_bc = np.repeat(bias_c.reshape(M16, 128).T[:, :, None], B, axis=2).reshape(128, 256)
    woutT = np.ascontiguousarray(
        W_out.T.reshape(8, 128, C).transpose(1, 0, 2).reshape(128, 8 * C)).astype(BF)
    bout_bc = np.tile(b_out[None, :].astype(np.float32), (T, 1))
    ident = np.eye(128, dtype=np.float32).astype(BF)
    identf = np.eye(128, dtype=np.float32)

    shared = {
        "wihT_a": wih_a.astype(BF), "whhT": whhT, "wcellT": wcellT,
        "bcell_bc": bcell_bc.astype(np.float32), "woutT": woutT,
        "bout_bc": bout_bc, "ident": ident, "identf": identf,
    }
    maps = []
    for core in range(NCORES):
        cb = condition[core * B:(core + 1) * B, :n_steps, :]  # [16, n, 32]
        ca = np.ones((C + 1, n_steps * B), np.float32)
        ca[0:C] = cb.transpose(2, 1, 0).reshape(C, n_steps * B)  # col = n*16+b
        m = dict(shared)
        m["cond_aT"] = ca.astype(BF)
        maps.append(m)
    return maps


_NC_CACHE = {}
LAST_RESULT = None


def _ensure_ntff_hook():
    """The agent image's antenv lacks axon_hooks; provide it and register the
    ctypes NTFF profiling hook so trace=True works under axon."""
    import types
    if "antenv.axon_hooks" in sys.modules:
        return
    mod = types.ModuleType("antenv.axon_hooks")
    _h = [None]
    mod.set_axon_ntff_profile_hook = lambda h: _h.__setitem__(0, h)
    mod.get_axon_ntff_profile_hook = lambda: _h[0]
    sys.modules["antenv.axon_hooks"] = mod
    if "/root/.axon_site" not in sys.path:
        sys.path.insert(0, "/root/.axon_site")
    from trn_agent_boot.trn_boot import _ntff_profile_via_ctypes
    mod.set_axon_ntff_profile_hook(_ntff_profile_via_ctypes("/opt/axon/libaxon_pjrt.so"))


def kernel(_trace=False, **inputs):
    global LAST_RESULT
    if _trace:
        try:
            _ensure_ntff_hook()
        except Exception as e:
            print("ntff hook setup failed:", e)
    inputs = {k: np.asarray(v) for k, v in inputs.items()}
    n_steps = N_STEPS
    if n_steps not in _NC_CACHE:
        _NC_CACHE[n_steps] = build_program(n_steps)
    nc = _NC_CACHE[n_steps]
    maps = prep_inputs(**inputs, n_steps=n_steps)
    res = run_bass_kernel_spmd(nc, maps, list(range(NCORES)), trace=_trace)
    LAST_RESULT = res
    out = np.concatenate([np.asarray(res.results[i]["out"], dtype=np.float32)
                          for i in range(NCORES)], axis=0)
    return out


# revision 15
# speedup vs baseline: 1.0381x; 1.0381x over previous
"""Trainium2 Bass kernel for nn_CCGGenerator (LSTM encoder + attention decoder).

Sharding: data-parallel, batch 128 -> 16 per core across 8 cores.
All weights replicated. Self-contained; everything hardcoded.

v2 design (per core, B=16):
- Encoder gates computed transposed: gates.T [2048, 16] as 16 PSUM cols-of-16
  split into TWO per-group PSUM tiles [128, 128] (group g = hidden half).
  Gate m-chunk order is re-packed in prep to
    [G0: i0 i1 f0 f1 o0 o1 g0 g1 | G1: i2 i3 f2 f3 o2 o3 g2 g3]
  so each group's gates are contiguous 128 PSUM cols: [i(32) f(32) o(32) g(32)].
- g-gate rows are pre-scaled x2 so tanh(x) = 2*sigmoid(2x)-1 turns the whole
  gate nonlinearity into ONE sigmoid over 128 cols + a tiny affine on 32 cols.
- Tail per group (8 wide ops instead of ~23 narrow ones):
    DVE  add   ga = gps_g + xg_g            [128,128]
    ACT  sigm  ga = sigmoid(ga)             [128,128] (g cols pre-scaled)
    DVE  affn  gt = 2*sigm-1                [128,32]
    DVE  ig    = i * gt                     [128,32]
    GPS  fc    = f * c_prev                 [128,32]   (parallel w/ DVE)
    DVE  c     = ig + fc                    [128,32]
    ACT  th    = tanh(c)                    [128,32]
    DVE  h     = o * th -> cl_sb (bf16)     [128,32]
- Software pipeline: PE emits S1=G0 MMs, S2=G1 MMs; tail(G0) overlaps S2 and
  next step's S1 (which only needs h chunks k=0,1 produced by tail(G0)).
- xg = x @ Wih.T + bias precomputed per 8-step chunk, stored s-major
  [128, s*256 + m*16 + b] so the tail add is a flat 2-level AP.
- Decoder: same structure, c = i*g (h0=c0=0 per reference), bias instead of xg.
- Attention per b unchanged from v1 (scores via strided CL reads, PE
  transposes for CL_b, ctx matmul, fused leaky-relu output GEMM).
"""
import sys
sys.path.insert(0, "/opt/trn_rl_repo")

import numpy as np
import ml_dtypes
from contextlib import ExitStack

import concourse.bass as bass
import concourse.tile as tile
from concourse import bacc, mybir
from concourse.bass_utils import run_bass_kernel_spmd

F32 = mybir.dt.float32
BF16 = mybir.dt.bfloat16
AF = mybir.ActivationFunctionType
OP = mybir.AluOpType
BF = ml_dtypes.bfloat16

NCORES = 8
B = 16          # batch per core
N_STEPS = 1024  # encoder sequence length
SCH = 8         # steps per xg chunk
C = 32
H = 512
G = 2048        # 4H
T = 24
KH = 4          # hidden 128-chunks
M16 = 16        # gate-dim 128-chunks


def build_program(n_steps=N_STEPS):
    nch = n_steps // SCH
    nac = n_steps // 128  # attention n-chunks
    nc = bacc.Bacc("TRN2", target_bir_lowering=False, debug=False,
                   num_devices=NCORES)

    p_cond = nc.declare_dram_parameter("cond_aT", [C + 1, n_steps * B], BF16, isOutput=False)
    p_wih = nc.declare_dram_parameter("wihT_a", [C + 1, G], BF16, isOutput=False)
    p_whh = nc.declare_dram_parameter("whhT", [128, 64 * 128], BF16, isOutput=False)
    p_wcell = nc.declare_dram_parameter("wcellT", [128, 64 * 128], BF16, isOutput=False)
    p_bcell = nc.declare_dram_parameter("bcell_bc", [128, 256], BF16, isOutput=False)
    p_wout = nc.declare_dram_parameter("woutT", [128, 8 * C], BF16, isOutput=False)
    p_bout = nc.declare_dram_parameter("bout_bc", [T, C], F32, isOutput=False)
    p_id = nc.declare_dram_parameter("ident", [128, 128], BF16, isOutput=False)
    p_out = nc.declare_dram_parameter("out", [B, T, C], F32, isOutput=True)

    with tile.TileContext(nc) as tc, ExitStack() as ctx:
        const = ctx.enter_context(tc.tile_pool(name="const", bufs=1))

        cl_sb = const.tile([128, n_steps * KH * B], BF16, tag="cl_sb")  # 16 MB
        hd_sb = const.tile([128, T * KH * B], BF16, tag="hd_sb")       # t-major
        wout_sb = const.tile([128, 8 * C], BF16, tag="wout_sb")
        nc.sync.dma_start(wout_sb[:], p_wout[:])
        bout_sb = const.tile([T, C], F32, tag="bout_sb")
        nc.sync.dma_start(bout_sb[:], p_bout[:])
        id_sb = const.tile([128, 128], BF16, tag="id_sb")
        nc.sync.dma_start(id_sb[:], p_id[:])
        bcell_sb = const.tile([128, 256], BF16, tag="bcell_sb")
        nc.sync.dma_start(bcell_sb[:], p_bcell[:])

        cl3 = cl_sb[:].rearrange("p (n k b) -> p n k b", k=KH, b=B)
        hd_v = hd_sb[:].rearrange("p (t k b) -> p t k b", k=KH, b=B)

        def lstm_tail(g, gps_g, xg_g, c_f, h_out, st_pool, ga_pool, fc_pool):
            """Per-group tail. gps_g [128,128] PSUM (cols i f o gt, 32 each).
            xg_g: [128,128] AP to add (xg slice or bias slice).
            c_f: persistent cell state [128, 64] or None (decoder: c = i*g).
            h_out: [128,32] AP (bf16 dest)."""
            ga = ga_pool.tile([128, 128], F32, tag=f"ga{g}")
            nc.scalar.activation(ga[:, 0:96], gps_g[:, 0:96], AF.Sigmoid)
            nc.scalar.activation(ga[:, 96:128], gps_g[:, 96:128], AF.Tanh)
            ig = st_pool.tile([128, 32], F32, tag=f"ig{g}")
            if c_f is not None:
                fc = fc_pool.tile([128, 32], F32, tag=f"fc{g}")
                nc.gpsimd.tensor_tensor(fc[:], ga[:, 32:64],
                                        c_f[:, g * 32:(g + 1) * 32], op=OP.mult)
                nc.vector.tensor_tensor(ig[:], ga[:, 0:32], ga[:, 96:128], op=OP.mult)
                nc.vector.tensor_tensor(c_f[:, g * 32:(g + 1) * 32], ig[:], fc[:],
                                        op=OP.add)
                csrc = c_f[:, g * 32:(g + 1) * 32]
            else:
                nc.vector.tensor_tensor(ig[:], ga[:, 0:32], ga[:, 96:128], op=OP.mult)
                csrc = ig[:]
            th = st_pool.tile([128, 32], F32, tag=f"th{g}")
            nc.scalar.activation(th[:], csrc, AF.Tanh)
            nc.vector.tensor_tensor(h_out, ga[:, 64:96], th[:], op=OP.mult)

        # ---------------- encoder + decoder (shared gate/state pools) ----------------
        rec_pools = ExitStack()
        gps_pool = rec_pools.enter_context(tc.tile_pool(name="gps", bufs=2, space="PSUM"))
        ga_pool = rec_pools.enter_context(tc.tile_pool(name="gtmp", bufs=2))
        st_pool = rec_pools.enter_context(tc.tile_pool(name="st", bufs=2))
        fc_pool = rec_pools.enter_context(tc.tile_pool(name="fcp", bufs=2))

        def step_mms(gps2, whh_like, hin, addv):
            """Emit the gate MMs for one step, grouped G0 then G1. Each
            group's psum bank holds ONE accumulation group: an identity
            matmul first injects xg (or the bias) over all 128 cols and opens
            the group, then the 32 Whh MMs accumulate in k-major order (so
            the k{0,1} half of step t+1 only depends on h chunks 0,1, i.e.
            tail g0 of step t)."""
            for g in (0, 1):
                nc.tensor.matmul(gps2[g][:], id_sb[:],
                                 addv(g), start=True, stop=False)
                for k in (0, 1):
                    for mj in range(8):
                        m = g * 8 + mj
                        nc.tensor.matmul(
                            gps2[g][:, bass.ts(mj, B)],
                            whh_like[:, bass.ts(m * KH + k, 128)],
                            hin(k), start=False, stop=False)
            for g in (0, 1):
                for k in (2, 3):
                    for mj in range(8):
                        m = g * 8 + mj
                        nc.tensor.matmul(
                            gps2[g][:, bass.ts(mj, B)],
                            whh_like[:, bass.ts(m * KH + k, 128)],
                            hin(k), start=False,
                            stop=(k == 3 and mj == 7))

        with tc.tile_pool(name="enc", bufs=1) as enc_pool, \
             tc.tile_pool(name="xg", bufs=2) as xg_pool, \
             tc.tile_pool(name="xg_ps", bufs=2, space="PSUM") as xg_ps_pool:
            whh_sb = enc_pool.tile([128, 64 * 128], BF16, tag="whh_sb")
            nc.sync.dma_start(whh_sb[:], p_whh[:])
            wih_sb = enc_pool.tile([C + 1, G], BF16, tag="wih_sb")
            nc.sync.dma_start(wih_sb[:], p_wih[:])
            c_f = enc_pool.tile([128, KH * B], F32, tag="c_f")
            h0 = enc_pool.tile([128, KH * B], BF16, tag="h0")
            nc.any.memset(c_f[:], 0.0)
            nc.any.memset(h0[:], 0.0)

            def fetch_cond(j):
                t = xg_pool.tile([C + 1, SCH * B], BF16, tag="cond_ch")
                nc.sync.dma_start(t[:], p_cond[:, bass.ts(j, SCH * B)])
                return t

            def xg_alloc():
                t = xg_pool.tile([128, SCH * M16 * B], BF16, tag="xg_sb")
                return t, t[:].rearrange("p (s m b) -> p s m b", s=SCH, m=M16)

            def xg_compute(xg_v, cond_t, mpair):
                """Two gate m-chunks (2*mpair, 2*mpair+1) of xg for a whole
                8-step chunk: two MMs into one PSUM tile, one DVE copy out
                (GPSIMD cannot read PSUM)."""
                xps = xg_ps_pool.tile([128, 2 * SCH * B], F32, tag="xps")
                for i in (0, 1):
                    nc.tensor.matmul(xps[:, bass.ts(i, SCH * B)],
                                     wih_sb[:, bass.ts(2 * mpair + i, 128)],
                                     cond_t[:], start=True, stop=True)
                nc.vector.tensor_copy(
                    xg_v[:, :, 2 * mpair:2 * mpair + 2, :],
                    xps[:].rearrange("p (m s b) -> p s m b", m=2, s=SCH))

            # prologue: cond 0,1 in flight; xg[0] computed in a burst
            cond_cur = fetch_cond(0)
            cond_nxt = fetch_cond(1) if nch > 1 else None
            xg_cur, xgv_cur = xg_alloc()
            for mp in range(M16 // 2):
                xg_compute(xgv_cur, cond_cur, mp)

            for chv in range(nch):
                if chv + 2 < nch:
                    cond_fut = fetch_cond(chv + 2)
                else:
                    cond_fut = None
                if chv + 1 < nch:
                    xg_nxt, xgv_nxt = xg_alloc()
                for s in range(SCH):
                    n = chv * SCH + s
                    hin = (lambda k: h0[:, bass.ts(k, B)]) if n == 0 else \
                        (lambda k, _n=n: cl_sb[:, (_n - 1) * 64 + k * B:(_n - 1) * 64 + (k + 1) * B])
                    gps2 = [gps_pool.tile([128, 128], F32, tag=f"gps{g}",
                                           name=f"gps{g}") for g in (0, 1)]
                    step_mms(gps2, whh_sb, hin,
                             lambda g, _s=s: xg_cur[:, _s * 256 + g * 128:_s * 256 + (g + 1) * 128])
                    # next chunk's xg precompute: fills the PE's wait-for-h gap
                    if chv + 1 < nch:
                        xg_compute(xgv_nxt, cond_nxt, s)
                    for g in (0, 1):
                        lstm_tail(g, gps2[g][:], None,
                                  c_f,
                                  cl_sb[:, n * 64 + g * 32:n * 64 + (g + 1) * 32],
                                  st_pool, ga_pool, fc_pool)
                if chv + 1 < nch:
                    xg_cur, xgv_cur = xg_nxt, xgv_nxt
                    cond_cur, cond_nxt = cond_nxt, cond_fut

        # ---------------- decoder ----------------
        with tc.tile_pool(name="dec", bufs=1) as dec_pool:
            wcell_sb = dec_pool.tile([128, 64 * 128], BF16, tag="wcell_sb")
            nc.sync.dma_start(wcell_sb[:], p_wcell[:])
            for t in range(T):
                if t == 0:
                    hin = lambda k: cl_sb[:, (n_steps - 1) * 64 + k * B:(n_steps - 1) * 64 + (k + 1) * B]
                else:
                    hin = lambda k, _t=t: hd_sb[:, (_t - 1) * 64 + k * B:(_t - 1) * 64 + (k + 1) * B]
                gps2 = [gps_pool.tile([128, 128], F32, tag=f"gps{g}",
                                       name=f"gps{g}") for g in (0, 1)]
                step_mms(gps2, wcell_sb, hin,
                         lambda g: bcell_sb[:, g * 128:(g + 1) * 128])
                for g in (0, 1):
                    lstm_tail(g, gps2[g][:], None,
                              None,
                              hd_sb[:, t * 64 + g * 32:t * 64 + (g + 1) * 32],
                              st_pool, ga_pool, fc_pool)
        rec_pools.close()

        # ---------------- attention + output, per batch ----------------
        with tc.tile_pool(name="att_fix", bufs=2) as att_fix, \
             tc.tile_pool(name="scr_ps", bufs=1, space="PSUM") as scr_ps_pool, \
             tc.tile_pool(name="tp_ps", bufs=2, space="PSUM") as tp_ps_pool, \
             tc.tile_pool(name="ctx_ps", bufs=2, space="PSUM") as ctx_ps_pool:
            for b in range(B):
                # scores [24, n]: lhsT = hd strided, rhs = cl_sb strided (CL.T native)
                scr = scr_ps_pool.tile([T, n_steps], F32, tag="scr")
                scn = min(512, n_steps)
                for k in range(KH):
                    for j in range(n_steps // scn):
                        rhs = cl3[:, j * scn:(j + 1) * scn, k, b]
                        nc.tensor.matmul(scr[:, bass.ts(j, scn)], hd_v[:, :, k, b],
                                         rhs, start=(k == 0), stop=(k == KH - 1))
                nmx = att_fix.tile([T, 1], F32, tag="nmx")
                nc.vector.reduce_max(nmx[:], scr[:], axis=mybir.AxisListType.X, negate=True)
                ex = att_fix.tile([T, n_steps], F32, tag="ex")
                sm = att_fix.tile([T, 1], F32, tag="sm")
                nc.scalar.activation(ex[:], scr[:], AF.Exp, bias=nmx[:], accum_out=sm[:])
                rc = att_fix.tile([T, 1], F32, tag="rc")
                nc.vector.reciprocal(rc[:], sm[:])
                cof = att_fix.tile([T, n_steps], BF16, tag="cof")
                nc.vector.tensor_scalar(cof[:], ex[:], rc[:], None, op0=OP.mult)
                # coeff.T [n, 24] via PE transposes
                cT = att_fix.tile([128, nac * T], BF16, tag="cT")
                for j in range(nac):
                    tp = tp_ps_pool.tile([128, 128], BF16, tag="tp")
                    nc.tensor.transpose(tp[:, 0:T], cof[:, bass.ts(j, 128)], id_sb[0:T, 0:T])
                    nc.vector.tensor_copy(cT[:, bass.ts(j, T)], tp[:, 0:T])
                # CL_b n-partitioned tiles via PE transposes
                clb = att_fix.tile([128, nac * KH * 128], BF16, tag="clb")
                for j in range(nac):
                    for k in range(KH):
                        tpc = tp_ps_pool.tile([128, 128], BF16, tag="tp")
                        nc.tensor.transpose(tpc[:], cl3[:, j * 128:(j + 1) * 128, k, b],
                                            id_sb[:, :])
                        nc.vector.tensor_copy(clb[:, bass.ts(j * KH + k, 128)], tpc[:])
                # ctx.T [512, 24]
                ctxp = ctx_ps_pool.tile([128, KH * T], F32, tag="ctxp")
                for k in range(KH):
                    for j in range(nac):
                        nc.tensor.matmul(ctxp[:, bass.ts(k, T)],
                                         clb[:, bass.ts(j * KH + k, 128)],
                                         cT[:, bass.ts(j, T)],
                                         start=(j == 0), stop=(j == nac - 1))
                # out [24, 32]
                ob_ps = scr_ps_pool.tile([T, C], F32, tag="ob_ps")
                for jj in range(8):
                    lr = att_fix.tile([128, T], BF16, tag="lr")
                    src = hd_v[:, :, jj, b] if jj < KH else ctxp[:, bass.ts(jj - KH, T)]
                    nc.scalar.activation(lr[:], src, AF.Lrelu, alpha=0.01)
                    nc.tensor.matmul(ob_ps[:], lr[:], wout_sb[:, bass.ts(jj, C)],
                                     start=(jj == 0), stop=(jj == 7))
                ob = att_fix.tile([T, C], F32, tag="ob")
                nc.vector.tensor_tensor(ob[:], ob_ps[:], bout_sb[:], op=OP.add)
                nc.sync.dma_start(p_out[b], ob[:])
    nc.compile()
    return nc


# m-chunk permutation: new m-position -> original gate chunk index (of 16).
# Original chunks: i=0..3, f=4..7, g=8..11, o=12..15 (PyTorch i,f,g,o order).
# New order: [G0: i0 i1 f0 f1 o0 o1 g0 g1 | G1: i2 i3 f2 f3 o2 o3 g2 g3]
M_PERM = [0, 1, 4, 5, 12, 13, 8, 9,
          2, 3, 6, 7, 14, 15, 10, 11]


def _reorder_cols(Wt):
    """Wt [*, 2048]: permute gate columns into the new m-chunk order."""
    Wn = Wt.reshape(Wt.shape[0], 16, 128)[:, M_PERM, :]
    return np.ascontiguousarray(Wn).reshape(Wt.shape[0], 2048)


def prep_inputs(condition, Wih_enc, Whh_enc, bih_enc, bhh_enc,
                Wih_cell, Whh_cell, bih_cell, bhh_cell, W_out, b_out,
                n_steps=N_STEPS):
    def tile64(Wt):  # [512, 2048] -> [128, 64*128], col block m*4+k
        return np.ascontiguousarray(
            Wt.reshape(KH, 128, M16, 128).transpose(1, 2, 0, 3).reshape(128, 64 * 128)).astype(BF)

    whhT = tile64(_reorder_cols(Whh_enc.T.astype(np.float32)))
    wcellT = tile64(_reorder_cols(Wih_cell.T.astype(np.float32)))
    wih_a = np.zeros((C + 1, G), np.float32)
    wih_a[0:C] = Wih_enc.T
    wih_a[C] = bih_enc + bhh_enc
    wih_a = _reorder_cols(wih_a)
    bias_c = _reorder_cols((bih_cell + bhh_cell).astype(np.float32)[None, :])[0]
    bcell_bc = np.repeat(bias_c.reshape(M16, 128).T[:, :, None], B, axis=2).reshape(128, 256)
    woutT = np.ascontiguousarray(
        W_out.T.reshape(8, 128, C).transpose(1, 0, 2).reshape(128, 8 * C)).astype(BF)
    bout_bc = np.tile(b_out[None, :].astype(np.float32), (T, 1))
    ident = np.eye(128, dtype=np.float32).astype(BF)

    shared = {
        "wihT_a": wih_a.astype(BF), "whhT": whhT, "wcellT": wcellT,
        "bcell_bc": bcell_bc.astype(BF), "woutT": woutT,
        "bout_bc": bout_bc, "ident": ident,
    }
    maps = []
    for core in range(NCORES):
        cb = condition[core * B:(core + 1) * B, :n_steps, :]  # [16, n, 32]
        ca = np.ones((C + 1, n_steps * B), np.float32)
        ca[0:C] = cb.transpose(2, 1, 0).reshape(C, n_steps * B)  # col = n*16+b
        m = dict(shared)
        m["cond_aT"] = ca.astype(BF)
        maps.append(m)
    return maps


_NC_CACHE = {}
LAST_RESULT = None


def _ensure_ntff_hook():
    """The agent image's antenv lacks axon_hooks; provide it and register the
    ctypes NTFF profiling hook so trace=True works under axon."""
    import types
    if "antenv.axon_hooks" in sys.modules:
        return
    mod = types.ModuleType("antenv.axon_hooks")
    _h = [None]
    mod.set_axon_ntff_profile_hook = lambda h: _h.__setitem__(0, h)
    mod.get_axon_ntff_profile_hook = lambda: _h[0]
    sys.modules["antenv.axon_hooks"] = mod
    if "/root/.axon_site" not in sys.path:
        sys.path.insert(0, "/root/.axon_site")
    from trn_agent_boot.trn_boot import _ntff_profile_via_ctypes
    mod.set_axon_ntff_profile_hook(_ntff_profile_via_ctypes("/opt/axon/libaxon_pjrt.so"))


def kernel(_trace=False, **inputs):
    global LAST_RESULT
    if _trace:
        try:
            _ensure_ntff_hook()
        except Exception as e:
            print("ntff hook setup failed:", e)
    inputs = {k: np.asarray(v) for k, v in inputs.items()}
    n_steps = N_STEPS
    if n_steps not in _NC_CACHE:
        _NC_CACHE[n_steps] = build_program(n_steps)
    nc = _NC_CACHE[n_steps]
    maps = prep_inputs(**inputs, n_steps=n_steps)
    res = run_bass_kernel_spmd(nc, maps, list(range(NCORES)), trace=_trace)
    LAST_RESULT = res
    out = np.concatenate([np.asarray(res.results[i]["out"], dtype=np.float32)
                          for i in range(NCORES)], axis=0)
    return out
